# revision 1
# baseline (speedup 1.0000x reference)
"""MQA attention kernel for Trainium2, 8-core SPMD.

Problem: Q [2, 8, 2048, 64] fp32, K/V [2, 1, 2048, 64] fp32 (shared head).
out[b,h,q,:] = softmax(Q[b,h,q,:] @ K[b,0]^T / 8) @ V[b,0].

Sharding: 16 (b,h) pairs over 8 cores -> core c handles b = c//4,
heads 2*(c%4), 2*(c%4)+1 (both heads share one K/V slice).

Per-core kernel (matmuls fp16, accumulation fp32 in PSUM):
  - One SWDGE cast-DMA packs Q[h0]|Q[h1] as the column halves of an HBM
    scratch [S, 128] fp16; one XBAR transpose-DMA turns that into
    QT [128, S] (partitions 0-63 = head0^T, 64-127 = head1^T).
  - K is cast with a broadcast source into both halves of its scratch ->
    KT [128, S] holds K^T replicated on both partition halves.
  - MM1 (scores^T): per k-tile kt, two matmuls on different PE row
    groups (contract=64) compute S_T[kt] [128k, 512q] for head0 (rows
    0-63) and head1 (rows 64-127) concurrently.
  - ACT exp on the 2-bank PSUM group [128, 2, 512] -> P^T fp16 in SBUF
    (1/sqrt(D) folded into the activation's free affine).
  - MM2: out^T[h] [65, 512] += V_aug[kt]^T @ P^T[kt,h]; V_aug carries a
    65th all-ones column so row 64 accumulates the softmax denominator.
  - PE-transpose out^T 128-column slices -> [128q, 65], DVE reciprocal +
    per-partition tensor_scalar mult to normalize, DMA out.
"""

import numpy as np

import concourse.bass as bass
import concourse.bacc as bacc
import concourse.mybir as mybir
import concourse.tile as tile
from concourse.bass_utils import run_bass_kernel_spmd
from concourse.masks import make_identity

F32 = mybir.dt.float32
F16 = mybir.dt.float16

B, H, S, D = 2, 8, 2048, 64
HPC = 2            # heads per core
NCORES = 8
QB = 512           # query block (PSUM bank free-dim limit for fp32)
NQB = S // QB      # 4
KT_TILE = 128      # keys per k-tile (PE contract partition limit)
NKT = S // KT_TILE # 16
SCALE = 1.0 / np.sqrt(np.float32(D))  # 0.125


def build_nc():
    nc = bacc.Bacc(None)
    Qd = nc.declare_dram_parameter("q", [HPC, S, D], F32, isOutput=False)
    Kd = nc.declare_dram_parameter("k", [S, D], F32, isOutput=False)
    Vd = nc.declare_dram_parameter("v", [S, D], F32, isOutput=False)
    Od = nc.declare_dram_parameter("o", [HPC, S, D], F32, isOutput=True)

    with tile.TileContext(nc) as tc:
        with (
            tc.tile_pool(name="const", bufs=1) as constp,
            tc.tile_pool(name="qk", bufs=1) as qkp,
            tc.tile_pool(name="vt", bufs=1) as vp,
            tc.tile_pool(name="pt", bufs=4) as ptp,
            tc.tile_pool(name="ot", bufs=2) as otp,
            tc.tile_pool(name="outsb", bufs=3) as outp,
            tc.tile_pool(name="rec", bufs=3) as recp,
            tc.tile_pool(name="psS", bufs=2, space="PSUM") as psSp,
            tc.tile_pool(name="psO", bufs=1, space="PSUM") as psOp,
            tc.tile_pool(name="psT", bufs=2, space="PSUM") as psTp,
        ):
            ident = constp.tile([128, 128], F32)
            make_identity(nc, ident[:])
            ident16 = constp.tile([128, 128], F16)
            make_identity(nc, ident16[:])

            # Prime the exp table load so the ~2.7us ACT_TABLE_LOAD overlaps
            # the input DMA phase instead of stalling the first real exp.
            dummy = constp.tile([128, 16], F32)
            nc.vector.memset(dummy[:], 0.0)
            nc.scalar.activation(dummy[:], dummy[:], mybir.ActivationFunctionType.Exp)

            # ---- input staging (all on-chip; transpose-DMA has a 1-wait
            # budget in walrus codegen, so PE-mode transposes are used
            # instead, in the window where PE is idle anyway) ----
            Qn = qkp.tile([128, HPC, NKT, D], F32, name="Qn")
            for h in range(HPC):
                nc.sync.dma_start(
                    out=Qn[:, h, :, :],
                    in_=Qd.ap()[h].rearrange("(t p) d -> p t d", p=128),
                )
            Kn = qkp.tile([128, NKT, D], F32, name="Kn")
            nc.sync.dma_start(
                out=Kn[:], in_=Kd.ap().rearrange("(t p) d -> p t d", p=128)
            )
            Qh = qkp.tile([128, HPC, NKT, D], F16, name="Qh")
            nc.vector.tensor_copy(Qh[:], Qn[:])
            Kh = qkp.tile([128, NKT, D], F16, name="Kh")
            nc.vector.tensor_copy(Kh[:], Kn[:])

            # V tiles [128k, kt, 65] fp16, 65th column = 1.0 (denominator).
            Vt = vp.tile([128, NKT, D + 1], F16)
            nc.gpsimd.dma_start(
                out=Vt[:, :, 0:D],
                in_=Vd.ap().rearrange("(t p) d -> p t d", p=128),
            )
            nc.vector.memset(Vt[:, :, D : D + 1], 1.0)

            # KT [128, S]: K^T on partitions 0-63 via PE transposes, then
            # replicated to 64-127 with one SBUF->SBUF DMA.
            # QT [128, S]: head0^T on partitions 0-63, head1^T on 64-127.
            KT = qkp.tile([128, S], F16, name="KT")
            QT = qkp.tile([128, S], F16, name="QT")
            for t in range(NKT):
                ts_ = slice(t * 128, (t + 1) * 128)
                psk = psTp.tile([64, 128], F16, tag="pst")
                nc.tensor.transpose(psk[:], Kh[:, t, :], ident16[:])
                nc.vector.tensor_copy(KT[0:64, ts_], psk[:])
                psq = psTp.tile([128, 128], F16, tag="pst")
                for h in range(HPC):
                    nc.tensor.transpose(
                        psq[64 * h : 64 * (h + 1), :],
                        Qh[:, h, t, :],
                        ident16[:],
                        tile_position=(0, 64 * h),
                    )
                nc.vector.tensor_copy(QT[:, ts_], psq[:])
            nc.sync.dma_start(out=KT[64:128, :], in_=KT[0:64, :])

            # ---- main loop ----
            for qb in range(NQB):
                qs = slice(qb * QB, (qb + 1) * QB)
                ps_o = [psOp.tile([D + 1, QB], F32, name=f"psO{h}") for h in range(HPC)]
                for kt in range(NKT):
                    ks = slice(kt * KT_TILE, (kt + 1) * KT_TILE)
                    ps_s = psSp.tile([128, HPC, QB], F32)
                    for h in range(HPC):
                        nc.tensor.matmul(
                            ps_s[:, h, :],
                            lhsT=KT[64 * h : 64 * (h + 1), ks],
                            rhs=QT[64 * h : 64 * (h + 1), qs],
                            start=True,
                            stop=True,
                        )
                    pt = ptp.tile([128, HPC, QB], F16)
                    nc.scalar.activation(
                        pt[:],
                        ps_s[:],
                        mybir.ActivationFunctionType.Exp,
                        scale=float(SCALE),
                    )
                    for h in range(HPC):
                        nc.tensor.matmul(
                            ps_o[h][:],
                            lhsT=Vt[:, kt, :],
                            rhs=pt[:, h, :],
                            start=(kt == 0),
                            stop=(kt == NKT - 1),
                        )
                # ---- drain: transpose + normalize + store ----
                for h in range(HPC):
                    ot = otp.tile([D + 1, QB], F32)
                    nc.vector.tensor_copy(ot[:], ps_o[h][:])
                    ps_t = psTp.tile([128, QB // 128, D + 1], F32, tag="pst")
                    rec = recp.tile([128, QB // 128, 1], F32)
                    outsb = outp.tile([128, QB // 128, D], F32)
                    for j in range(QB // 128):
                        nc.tensor.transpose(
                            ps_t[:, j, :],
                            ot[:, j * 128 : (j + 1) * 128],
                            ident[0 : D + 1, 0 : D + 1],
                        )
                        nc.vector.reciprocal(rec[:, j, :], ps_t[:, j, D : D + 1])
                        nc.vector.tensor_scalar_mul(
                            outsb[:, j, :], ps_t[:, j, 0:D], rec[:, j, :]
                        )
                    nc.sync.dma_start(
                        out=Od.ap()[h, qs, :].rearrange("(j p) d -> p j d", p=128),
                        in_=outsb[:],
                    )
    nc.compile()
    return nc


_CACHED = {}


def _get_nc():
    if "nc" not in _CACHED:
        _CACHED["nc"] = build_nc()
    return _CACHED["nc"]


def _shard(Q, K, V):
    in_maps = []
    for c in range(NCORES):
        b = c // 4
        h0 = (c % 4) * HPC
        in_maps.append(
            {
                "q": np.ascontiguousarray(np.asarray(Q, np.float32)[b, h0 : h0 + HPC]),
                "k": np.ascontiguousarray(np.asarray(K, np.float32)[b, 0]),
                "v": np.ascontiguousarray(np.asarray(V, np.float32)[b, 0]),
            }
        )
    return in_maps


def kernel(Q, K, V, trace=False):
    nc = _get_nc()
    res = run_bass_kernel_spmd(nc, _shard(Q, K, V), list(range(NCORES)), trace=trace)
    _CACHED["last_result"] = res
    O = np.empty((B, H, S, D), np.float32)
    for c, r in enumerate(res.results):
        b = c // 4
        h0 = (c % 4) * HPC
        O[b, h0 : h0 + HPC] = r["o"]
    return O



# revision 5
# speedup vs baseline: 1.2889x; 1.2889x over previous
"""MQA attention kernel for Trainium2, 8-core SPMD.

Problem: Q [2, 8, 2048, 64] fp32, K/V [2, 1, 2048, 64] fp32 (shared head).
out[b,h,q,:] = softmax(Q[b,h,q,:] @ K[b,0]^T / 8) @ V[b,0].

Sharding: 16 (b,h) pairs over 8 cores -> core c handles b = c//4,
heads 2*(c%4), 2*(c%4)+1 (both heads share one K/V slice).

v2 structure (per core; matmuls fp16, accumulation fp32 in PSUM):
  - Staging via gpsimd cast-DMAs (fp32->fp16 in the DMA): K first, then
    the Q chunks pass 0 needs, then V, then the rest of Q.  PE transposes
    (fp16) build KT/QT; K^T is replicated to partitions 64-127 so the two
    heads' MM1s run concurrently on different PE row-quadrants.
  - One flat loop over i = (pass, kt) with MM1 emitted 2 iterations ahead
    of MM2 (scores triple-buffered in PSUM) so the PE never sits on the
    ACT->MM2 dependence.
  - ACT: exp on the [128, 2, 512] score tile (scale=1/8 folded in), fp16
    out.  This is the bound: ~1.3us per tile.
  - MM2: one matmul per kt covering both heads ([65, 2, 512] out) with the
    V_aug weights (65th all-ones column accumulates the denominator).
  - Drain per pass overlapped with the next pass's first MM1s: DVE copy
    psO->SBUF fp16, PE transpose (into a scores-pool PSUM slot), DVE
    reciprocal + per-partition scalar mult, DMA out.
"""

import numpy as np

import concourse.bass as bass
import concourse.bacc as bacc
import concourse.mybir as mybir
import concourse.tile as tile
from concourse.bass_utils import run_bass_kernel_spmd
from concourse.masks import make_identity

F32 = mybir.dt.float32
F16 = mybir.dt.float16

B, H, S, D = 2, 8, 2048, 64
HPC = 2            # heads per core
NCORES = 8
QB = 512           # query block (PSUM bank free-dim limit for fp32)
NQB = S // QB      # 4
KT_TILE = 128      # keys per k-tile (PE contract partition limit)
NKT = S // KT_TILE # 16
NIT = NQB * NKT    # 64 flattened (pass, kt) iterations
SCALE = 1.0 / np.sqrt(np.float32(D))  # 0.125
MERGED_MM2 = False  # one [65, 2, 512]-out matmul per kt (both heads)


def build_nc():
    nc = bacc.Bacc(None)
    Qd = nc.declare_dram_parameter("q", [HPC, S, D], F32, isOutput=False)
    Kd = nc.declare_dram_parameter("k", [S, D], F32, isOutput=False)
    Vd = nc.declare_dram_parameter("v", [S, D], F32, isOutput=False)
    Od = nc.declare_dram_parameter("o", [HPC, S, D], F32, isOutput=True)

    with tile.TileContext(nc) as tc:
        with (
            tc.tile_pool(name="const", bufs=1) as constp,
            tc.tile_pool(name="qk", bufs=1) as qkp,
            tc.tile_pool(name="vt", bufs=1) as vp,
            tc.tile_pool(name="pt", bufs=4) as ptp,
            tc.tile_pool(name="ot", bufs=2) as otp,
            tc.tile_pool(name="outsb", bufs=3) as outp,
            tc.tile_pool(name="rec", bufs=3) as recp,
            tc.tile_pool(name="psS", bufs=3, space="PSUM") as psSp,
            tc.tile_pool(name="psO", bufs=1, space="PSUM") as psOp,
        ):
            ident16 = constp.tile([128, 128], F16)
            make_identity(nc, ident16[:])

            # Prime the exp table load so the ~2.7us ACT_TABLE_LOAD overlaps
            # the input DMA phase instead of stalling the first real exp.
            dummy = constp.tile([128, 16], F32)
            nc.vector.memset(dummy[:], 0.0)
            nc.scalar.activation(dummy[:], dummy[:], mybir.ActivationFunctionType.Exp)

            # ---- input staging: gpsimd cast-DMAs (fp32 -> fp16), ordered so
            # pass 0 can start as early as possible ----
            Kh = qkp.tile([128, NKT, D], F16, name="Kh")
            nc.gpsimd.dma_start(
                out=Kh[:], in_=Kd.ap().rearrange("(t p) d -> p t d", p=128)
            )
            Qh = qkp.tile([128, HPC, NKT, D], F16, name="Qh")
            for h in range(HPC):
                nc.gpsimd.dma_start(
                    out=Qh[:, h, 0:4, :],
                    in_=Qd.ap()[h, 0:QB, :].rearrange("(t p) d -> p t d", p=128),
                )
            # V tiles [128k, kt, 65] fp16, 65th column = 1.0 (denominator).
            Vt = vp.tile([128, NKT, D + 1], F16)
            nc.gpsimd.dma_start(
                out=Vt[:, :, 0:D],
                in_=Vd.ap().rearrange("(t p) d -> p t d", p=128),
            )
            nc.gpsimd.memset(Vt[:, :, D : D + 1], 1.0)
            for h in range(HPC):
                nc.gpsimd.dma_start(
                    out=Qh[:, h, 4:NKT, :],
                    in_=Qd.ap()[h, QB:S, :].rearrange("(t p) d -> p t d", p=128),
                )

            # KT [128, S]: K^T on partitions 0-63 via PE transposes, then
            # replicated to 64-127 with one SBUF->SBUF DMA.
            # QT [128, S]: head0^T on partitions 0-63, head1^T on 64-127.
            KT = qkp.tile([128, S], F16, name="KT")
            QT = qkp.tile([128, S], F16, name="QT")

            def stage_k(t):
                ts_ = slice(t * 128, (t + 1) * 128)
                psk = psSp.tile([64, 128], F16, name="psk", tag="ps")
                nc.tensor.transpose(psk[:], Kh[:, t, :], ident16[:])
                nc.vector.tensor_copy(KT[0:64, ts_], psk[:])

            def stage_q(t):
                ts_ = slice(t * 128, (t + 1) * 128)
                psq = psSp.tile([128, 128], F16, name="psq", tag="ps")
                for h in range(HPC):
                    nc.tensor.transpose(
                        psq[64 * h : 64 * (h + 1), :],
                        Qh[:, h, t, :],
                        ident16[:],
                        tile_position=(0, 64 * h),
                    )
                nc.vector.tensor_copy(QT[:, ts_], psq[:])

            for t in range(NKT):
                stage_k(t)
            nc.sync.dma_start(out=KT[64:128, :], in_=KT[0:64, :])
            for t in range(4):
                stage_q(t)

            # ---- flattened main loop over (pass, kt) ----
            sc = {}    # i -> score psum tile
            pt = {}    # i -> prob sbuf tile
            ps_o = {}  # pass -> psO tile [65, 2, 512]

            def emit_mm1(i):
                if i >= NIT:
                    return
                p, kt = divmod(i, NKT)
                qs = slice(p * QB, (p + 1) * QB)
                ks = slice(kt * KT_TILE, (kt + 1) * KT_TILE)
                ps_s = psSp.tile([128, HPC, QB], F32, name="ps_s", tag="ps")
                for h in range(HPC):
                    nc.tensor.matmul(
                        ps_s[:, h, :],
                        lhsT=KT[64 * h : 64 * (h + 1), ks],
                        rhs=QT[64 * h : 64 * (h + 1), qs],
                        start=True,
                        stop=True,
                    )
                sc[i] = ps_s

            def emit_act(i):
                ptile = ptp.tile([128, HPC, QB], F16, name="ptile")
                nc.scalar.activation(
                    ptile[:],
                    sc.pop(i)[:],
                    mybir.ActivationFunctionType.Exp,
                    scale=float(SCALE),
                )
                pt[i] = ptile

            def emit_mm2(i):
                p, kt = divmod(i, NKT)
                if kt == 0:
                    ps_o[p] = psOp.tile([D + 1, HPC, QB], F32, name="psO", tag="psO")
                ptile = pt.pop(i)
                if MERGED_MM2:
                    nc.tensor.matmul(
                        ps_o[p][:],
                        lhsT=Vt[:, kt, :],
                        rhs=ptile[:],
                        start=(kt == 0),
                        stop=(kt == NKT - 1),
                    )
                else:
                    for h in range(HPC):
                        nc.tensor.matmul(
                            ps_o[p][:, h, :],
                            lhsT=Vt[:, kt, :],
                            rhs=ptile[:, h, :],
                            start=(kt == 0),
                            stop=(kt == NKT - 1),
                        )

            def emit_drain(p):
                qs = slice(p * QB, (p + 1) * QB)
                for h in range(HPC):
                    ot = otp.tile([D + 1, QB], F16)
                    nc.vector.tensor_copy(ot[:], ps_o[p][:, h, :])
                    ps_t = psSp.tile([128, QB // 128, D + 2], F16, name="ps_t", tag="ps")
                    for j in range(QB // 128):
                        nc.tensor.transpose(
                            ps_t[:, j, 0 : D + 1],
                            ot[:, j * 128 : (j + 1) * 128],
                            ident16[0 : D + 1, 0 : D + 1],
                        )
                    rec = recp.tile([128, QB // 128, 1], F32)
                    outsb = outp.tile([128, QB // 128, D], F32)
                    for j in range(QB // 128):
                        nc.vector.reciprocal(rec[:, j, :], ps_t[:, j, D : D + 1])
                        nc.vector.tensor_scalar_mul(
                            outsb[:, j, :], ps_t[:, j, 0:D], rec[:, j, :]
                        )
                    nc.sync.dma_start(
                        out=Od.ap()[h, qs, :].rearrange("(j p) d -> p j d", p=128),
                        in_=outsb[:],
                    )
                del ps_o[p]

            emit_mm1(0)
            emit_mm1(1)
            for i in range(NIT):
                p, kt = divmod(i, NKT)
                emit_mm1(i + 2)
                # Stage the remaining Q tiles during pass 0 (PE work that
                # fills dependence stalls while the pipeline warms up).
                if p == 0 and kt in (2, 3, 4, 5) and NQB > 1:
                    base = 4 + (kt - 2) * 3
                    for t in range(base, min(base + 3, NKT)):
                        stage_q(t)
                emit_act(i)
                emit_mm2(i)
                if kt == NKT - 1:
                    emit_drain(p)
    nc.compile()
    return nc


_CACHED = {}


def _get_nc():
    if "nc" not in _CACHED:
        _CACHED["nc"] = build_nc()
    return _CACHED["nc"]


def _shard(Q, K, V):
    in_maps = []
    for c in range(NCORES):
        b = c // 4
        h0 = (c % 4) * HPC
        in_maps.append(
            {
                "q": np.ascontiguousarray(np.asarray(Q, np.float32)[b, h0 : h0 + HPC]),
                "k": np.ascontiguousarray(np.asarray(K, np.float32)[b, 0]),
                "v": np.ascontiguousarray(np.asarray(V, np.float32)[b, 0]),
            }
        )
    return in_maps


def kernel(Q, K, V, trace=False):
    nc = _get_nc()
    res = run_bass_kernel_spmd(nc, _shard(Q, K, V), list(range(NCORES)), trace=trace)
    _CACHED["last_result"] = res
    O = np.empty((B, H, S, D), np.float32)
    for c, r in enumerate(res.results):
        b = c // 4
        h0 = (c % 4) * HPC
        O[b, h0 : h0 + HPC] = r["o"]
    return O


# revision 7
# speedup vs baseline: 1.3567x; 1.0526x over previous
"""MQA attention kernel for Trainium2, 8-core SPMD.

Problem: Q [2, 8, 2048, 64] fp32, K/V [2, 1, 2048, 64] fp32 (shared head).
out[b,h,q,:] = softmax(Q[b,h,q,:] @ K[b,0]^T / 8) @ V[b,0].

Sharding: 16 (b,h) pairs over 8 cores -> core c handles b = c//4,
heads 2*(c%4), 2*(c%4)+1 (both heads share one K/V slice).

v2 structure (per core; matmuls fp16, accumulation fp32 in PSUM):
  - Staging via gpsimd cast-DMAs (fp32->fp16 in the DMA): K first, then
    the Q chunks pass 0 needs, then V, then the rest of Q.  PE transposes
    (fp16) build KT/QT; K^T is replicated to partitions 64-127 so the two
    heads' MM1s run concurrently on different PE row-quadrants.
  - One flat loop over i = (pass, kt) with MM1 emitted 2 iterations ahead
    of MM2 (scores triple-buffered in PSUM) so the PE never sits on the
    ACT->MM2 dependence.
  - ACT: exp on the [128, 2, 512] score tile (scale=1/8 folded in), fp16
    out.  This is the bound: ~1.3us per tile.
  - MM2: one matmul per kt covering both heads ([65, 2, 512] out) with the
    V_aug weights (65th all-ones column accumulates the denominator).
  - Drain per pass overlapped with the next pass's first MM1s: DVE copy
    psO->SBUF fp16, PE transpose (into a scores-pool PSUM slot), DVE
    reciprocal + per-partition scalar mult, DMA out.
"""

import numpy as np

import concourse.bass as bass
import concourse.bacc as bacc
import concourse.mybir as mybir
import concourse.tile as tile
from concourse.bass_utils import run_bass_kernel_spmd
from concourse.masks import make_identity

F32 = mybir.dt.float32
F16 = mybir.dt.float16

B, H, S, D = 2, 8, 2048, 64
HPC = 2            # heads per core
NCORES = 8
QB = 512           # query block (PSUM bank free-dim limit for fp32)
NQB = S // QB      # 4
KT_TILE = 128      # keys per k-tile (PE contract partition limit)
NKT = S // KT_TILE # 16
NIT = NQB * NKT    # 64 flattened (pass, kt) iterations
SCALE = 1.0 / np.sqrt(np.float32(D))  # 0.125
MERGED_MM2 = False  # ISA caps matmul free dim at 512; keep per-head MM2s
FILLER_N = 2       # junk ldweights per kt keeping the PE DVFS ramped


def build_nc():
    nc = bacc.Bacc(None)
    Qd = nc.declare_dram_parameter("q", [HPC, S, D], F32, isOutput=False)
    Kd = nc.declare_dram_parameter("k", [S, D], F32, isOutput=False)
    Vd = nc.declare_dram_parameter("v", [S, D], F32, isOutput=False)
    Od = nc.declare_dram_parameter("o", [HPC, S, D], F32, isOutput=True)

    with tile.TileContext(nc) as tc:
        with (
            tc.tile_pool(name="const", bufs=1) as constp,
            tc.tile_pool(name="qk", bufs=1) as qkp,
            tc.tile_pool(name="vt", bufs=1) as vp,
            tc.tile_pool(name="pt", bufs=4) as ptp,
            tc.tile_pool(name="ot", bufs=2) as otp,
            tc.tile_pool(name="outsb", bufs=3) as outp,
            tc.tile_pool(name="rec", bufs=3) as recp,
            tc.tile_pool(name="psS", bufs=3, space="PSUM") as psSp,
            tc.tile_pool(name="psO", bufs=1, space="PSUM") as psOp,
        ):
            # ---- input staging: gpsimd cast-DMAs (fp32 -> fp16), ordered so
            # pass 0 can start as early as possible ----
            Kh = qkp.tile([128, NKT, D], F16, name="Kh")
            nc.gpsimd.dma_start(
                out=Kh[:], in_=Kd.ap().rearrange("(t p) d -> p t d", p=128)
            )
            Qh = qkp.tile([128, HPC, NKT, D], F16, name="Qh")
            for h in range(HPC):
                nc.gpsimd.dma_start(
                    out=Qh[:, h, 0:4, :],
                    in_=Qd.ap()[h, 0:QB, :].rearrange("(t p) d -> p t d", p=128),
                )
            ident16 = constp.tile([128, 128], F16)
            make_identity(nc, ident16[:])

            # Prime the exp table load so the ~2.7us ACT_TABLE_LOAD overlaps
            # the input DMA phase instead of stalling the first real exp.
            dummy = constp.tile([128, 16], F32)
            nc.vector.memset(dummy[:], 0.0)
            nc.scalar.activation(dummy[:], dummy[:], mybir.ActivationFunctionType.Exp)

            # V tiles [128k, kt, 65] fp16, 65th column = 1.0 (denominator).
            Vt = vp.tile([128, NKT, D + 1], F16)
            nc.gpsimd.dma_start(
                out=Vt[:, :, 0:D],
                in_=Vd.ap().rearrange("(t p) d -> p t d", p=128),
            )
            nc.gpsimd.memset(Vt[:, :, D : D + 1], 1.0)
            for h in range(HPC):
                nc.gpsimd.dma_start(
                    out=Qh[:, h, 4:NKT, :],
                    in_=Qd.ap()[h, QB:S, :].rearrange("(t p) d -> p t d", p=128),
                )

            # KT [128, S]: K^T on partitions 0-63 via PE transposes, then
            # replicated to 64-127 with one SBUF->SBUF DMA.
            # QT [128, S]: head0^T on partitions 0-63, head1^T on 64-127.
            KT = qkp.tile([128, S], F16, name="KT")
            QT = qkp.tile([128, S], F16, name="QT")

            def stage_k(t):
                ts_ = slice(t * 128, (t + 1) * 128)
                psk = psSp.tile([64, 128], F16, name="psk", tag="ps")
                nc.tensor.transpose(psk[:], Kh[:, t, :], ident16[:])
                nc.vector.tensor_copy(KT[0:64, ts_], psk[:])

            def stage_q(t):
                ts_ = slice(t * 128, (t + 1) * 128)
                psq = psSp.tile([128, 128], F16, name="psq", tag="ps")
                for h in range(HPC):
                    nc.tensor.transpose(
                        psq[64 * h : 64 * (h + 1), :],
                        Qh[:, h, t, :],
                        ident16[:],
                        tile_position=(0, 64 * h),
                    )
                nc.vector.tensor_copy(QT[:, ts_], psq[:])

            for t in range(NKT):
                stage_k(t)
            nc.sync.dma_start(out=KT[64:128, :], in_=KT[0:64, :])
            for t in range(4):
                stage_q(t)

            # ---- flattened main loop over (pass, kt) ----
            sc = {}    # i -> score psum tile
            pt = {}    # i -> prob sbuf tile
            ps_o = {}  # pass -> psO tile [65, 2, 512]

            def emit_mm1(i):
                if i >= NIT:
                    return
                p, kt = divmod(i, NKT)
                qs = slice(p * QB, (p + 1) * QB)
                ks = slice(kt * KT_TILE, (kt + 1) * KT_TILE)
                ps_s = psSp.tile([128, HPC, QB], F32, name="ps_s", tag="ps")
                for h in range(HPC):
                    nc.tensor.matmul(
                        ps_s[:, h, :],
                        lhsT=KT[64 * h : 64 * (h + 1), ks],
                        rhs=QT[64 * h : 64 * (h + 1), qs],
                        start=True,
                        stop=True,
                    )
                sc[i] = ps_s

            def emit_act(i):
                ptile = ptp.tile([128, HPC, QB], F16, name="ptile")
                nc.scalar.activation(
                    ptile[:],
                    sc.pop(i)[:],
                    mybir.ActivationFunctionType.Exp,
                    scale=float(SCALE),
                )
                pt[i] = ptile

            def emit_mm2(i):
                p, kt = divmod(i, NKT)
                if kt == 0:
                    ps_o[p] = psOp.tile([D + 1, HPC, QB], F32, name="psO", tag="psO")
                ptile = pt.pop(i)
                if MERGED_MM2:
                    nc.tensor.matmul(
                        ps_o[p][:],
                        lhsT=Vt[:, kt, :],
                        rhs=ptile[:],
                        start=(kt == 0),
                        stop=(kt == NKT - 1),
                    )
                else:
                    for h in range(HPC):
                        nc.tensor.matmul(
                            ps_o[p][:, h, :],
                            lhsT=Vt[:, kt, :],
                            rhs=ptile[:, h, :],
                            start=(kt == 0),
                            stop=(kt == NKT - 1),
                        )

            def emit_drain(p):
                qs = slice(p * QB, (p + 1) * QB)
                ots, psts, recs, outs = [], [], [], []
                for h in range(HPC):
                    ot = otp.tile([D + 1, QB], F16, name="ot")
                    nc.vector.tensor_copy(ot[:], ps_o[p][:, h, :])
                    ots.append(ot)
                for h in range(HPC):
                    ps_t = psSp.tile([128, QB // 128, D + 2], F16, name="ps_t", tag="ps")
                    for j in range(QB // 128):
                        nc.tensor.transpose(
                            ps_t[:, j, 0 : D + 1],
                            ots[h][:, j * 128 : (j + 1) * 128],
                            ident16[0 : D + 1, 0 : D + 1],
                        )
                    psts.append(ps_t)
                    rec = recp.tile([128, QB // 128, 1], F32, name="rec")
                    nc.vector.reciprocal(rec[:], ps_t[:, :, D : D + 1])
                    recs.append(rec)
                for h in range(HPC):
                    outsb = outp.tile([128, QB // 128, D], F32, name="outsb")
                    for j in range(QB // 128):
                        nc.vector.tensor_scalar_mul(
                            outsb[:, j, :], psts[h][:, j, 0:D], recs[h][:, j, :]
                        )
                    nc.sync.dma_start(
                        out=Od.ap()[h, qs, :].rearrange("(j p) d -> p j d", p=128),
                        in_=outsb[:],
                    )
                del ps_o[p]

            emit_mm1(0)
            emit_mm1(1)
            for i in range(NIT):
                p, kt = divmod(i, NKT)
                emit_mm1(i + 2)
                # Stage the remaining Q tiles during pass 0 (PE work that
                # fills dependence stalls while the pipeline warms up).
                if p == 0 and 2 <= kt <= 13 and NQB > 1:
                    stage_q(2 + kt)
                # Dependency-free PE filler: a junk weight load sized so the
                # PE stream stays gapless at the ACT-bound cadence.  Without
                # it the tensor engine's DVFS ramp (2.4GHz needs ~3us of
                # continuous busy) keeps resetting on the ACT->MM2 waits.
                for _ in range(FILLER_N):
                    nc.tensor.ldweights(KT[0:64, 0:128])
                emit_act(i)
                emit_mm2(i)
                if kt == NKT - 1:
                    emit_drain(p)
    nc.compile()
    return nc


_CACHED = {}


def _get_nc():
    if "nc" not in _CACHED:
        _CACHED["nc"] = build_nc()
    return _CACHED["nc"]


def _shard(Q, K, V):
    in_maps = []
    for c in range(NCORES):
        b = c // 4
        h0 = (c % 4) * HPC
        in_maps.append(
            {
                "q": np.ascontiguousarray(np.asarray(Q, np.float32)[b, h0 : h0 + HPC]),
                "k": np.ascontiguousarray(np.asarray(K, np.float32)[b, 0]),
                "v": np.ascontiguousarray(np.asarray(V, np.float32)[b, 0]),
            }
        )
    return in_maps


def kernel(Q, K, V, trace=False):
    nc = _get_nc()
    res = run_bass_kernel_spmd(nc, _shard(Q, K, V), list(range(NCORES)), trace=trace)
    _CACHED["last_result"] = res
    O = np.empty((B, H, S, D), np.float32)
    for c, r in enumerate(res.results):
        b = c // 4
        h0 = (c % 4) * HPC
        O[b, h0 : h0 + HPC] = r["o"]
    return O


# revision 8
# speedup vs baseline: 1.3606x; 1.0028x over previous
"""MQA attention kernel for Trainium2, 8-core SPMD.

Problem: Q [2, 8, 2048, 64] fp32, K/V [2, 1, 2048, 64] fp32 (shared head).
out[b,h,q,:] = softmax(Q[b,h,q,:] @ K[b,0]^T / 8) @ V[b,0].

Sharding: 16 (b,h) pairs over 8 cores -> core c handles b = c//4,
heads 2*(c%4), 2*(c%4)+1 (both heads share one K/V slice).

v2 structure (per core; matmuls fp16, accumulation fp32 in PSUM):
  - Staging via gpsimd cast-DMAs (fp32->fp16 in the DMA): K first, then
    the Q chunks pass 0 needs, then V, then the rest of Q.  PE transposes
    (fp16) build KT/QT; K^T is replicated to partitions 64-127 so the two
    heads' MM1s run concurrently on different PE row-quadrants.
  - One flat loop over i = (pass, kt) with MM1 emitted 2 iterations ahead
    of MM2 (scores triple-buffered in PSUM) so the PE never sits on the
    ACT->MM2 dependence.
  - ACT: exp on the [128, 2, 512] score tile (scale=1/8 folded in), fp16
    out.  This is the bound: ~1.3us per tile.
  - MM2: one matmul per kt covering both heads ([65, 2, 512] out) with the
    V_aug weights (65th all-ones column accumulates the denominator).
  - Drain per pass overlapped with the next pass's first MM1s: DVE copy
    psO->SBUF fp16, PE transpose (into a scores-pool PSUM slot), DVE
    reciprocal + per-partition scalar mult, DMA out.
"""

import numpy as np

import concourse.bass as bass
import concourse.bacc as bacc
import concourse.mybir as mybir
import concourse.tile as tile
from concourse.bass_utils import run_bass_kernel_spmd
from concourse.masks import make_identity

F32 = mybir.dt.float32
F16 = mybir.dt.float16

B, H, S, D = 2, 8, 2048, 64
HPC = 2            # heads per core
NCORES = 8
QB = 512           # query block (PSUM bank free-dim limit for fp32)
NQB = S // QB      # 4
KT_TILE = 128      # keys per k-tile (PE contract partition limit)
NKT = S // KT_TILE # 16
NIT = NQB * NKT    # 64 flattened (pass, kt) iterations
SCALE = 1.0 / np.sqrt(np.float32(D))  # 0.125
MERGED_MM2 = False  # ISA caps matmul free dim at 512; keep per-head MM2s
FILLER_N = 2       # junk ldweights per kt keeping the PE DVFS ramped


def build_nc():
    nc = bacc.Bacc(None)
    Qd = nc.declare_dram_parameter("q", [HPC, S, D], F32, isOutput=False)
    Kd = nc.declare_dram_parameter("k", [S, D], F32, isOutput=False)
    Vd = nc.declare_dram_parameter("v", [S, D], F32, isOutput=False)
    Od = nc.declare_dram_parameter("o", [HPC, S, D], F32, isOutput=True)

    with tile.TileContext(nc) as tc:
        with (
            tc.tile_pool(name="const", bufs=1) as constp,
            tc.tile_pool(name="qk", bufs=1) as qkp,
            tc.tile_pool(name="vt", bufs=1) as vp,
            tc.tile_pool(name="pt", bufs=4) as ptp,
            tc.tile_pool(name="ot", bufs=2) as otp,
            tc.tile_pool(name="outsb", bufs=3) as outp,
            tc.tile_pool(name="rec", bufs=3) as recp,
            tc.tile_pool(name="psS", bufs=3, space="PSUM") as psSp,
            tc.tile_pool(name="psO", bufs=1, space="PSUM") as psOp,
        ):
            # ---- input staging.  The time-critical K and pass-0 Q go over the
            # sync HWDGE queue as fp32 + DVE casts (starts immediately); the
            # identity goes first on gpsimd so PE transposes are never blocked
            # on it; V and the rest of Q use gpsimd cast-DMAs in parallel. ----
            ident16 = constp.tile([128, 128], F16)
            make_identity(nc, ident16[:])

            Kn = qkp.tile([128, NKT, D], F32, name="Kn")
            nc.sync.dma_start(
                out=Kn[:], in_=Kd.ap().rearrange("(t p) d -> p t d", p=128)
            )
            Qn0 = qkp.tile([128, HPC, 4, D], F32, name="Qn0")
            for h in range(HPC):
                nc.sync.dma_start(
                    out=Qn0[:, h, :, :],
                    in_=Qd.ap()[h, 0:QB, :].rearrange("(t p) d -> p t d", p=128),
                )
            Kh = qkp.tile([128, NKT, D], F16, name="Kh")
            nc.vector.tensor_copy(Kh[:], Kn[:])
            Qh = qkp.tile([128, HPC, NKT, D], F16, name="Qh")
            nc.vector.tensor_copy(Qh[:, :, 0:4, :], Qn0[:])

            # Prime the exp table load so the ~2.7us ACT_TABLE_LOAD overlaps
            # the input DMA phase instead of stalling the first real exp.
            dummy = constp.tile([128, 16], F32)
            nc.vector.memset(dummy[:], 0.0)
            nc.scalar.activation(dummy[:], dummy[:], mybir.ActivationFunctionType.Exp)

            # V tiles [128k, kt, 65] fp16, 65th column = 1.0 (denominator).
            Vt = vp.tile([128, NKT, D + 1], F16)
            nc.gpsimd.dma_start(
                out=Vt[:, :, 0:D],
                in_=Vd.ap().rearrange("(t p) d -> p t d", p=128),
            )
            nc.gpsimd.memset(Vt[:, :, D : D + 1], 1.0)
            for h in range(HPC):
                nc.gpsimd.dma_start(
                    out=Qh[:, h, 4:NKT, :],
                    in_=Qd.ap()[h, QB:S, :].rearrange("(t p) d -> p t d", p=128),
                )

            # KT [128, S]: K^T on partitions 0-63 via PE transposes, then
            # replicated to 64-127 with one SBUF->SBUF DMA.
            # QT [128, S]: head0^T on partitions 0-63, head1^T on 64-127.
            KT = qkp.tile([128, S], F16, name="KT")
            QT = qkp.tile([128, S], F16, name="QT")

            def stage_k(t):
                ts_ = slice(t * 128, (t + 1) * 128)
                psk = psSp.tile([64, 128], F16, name="psk", tag="ps")
                nc.tensor.transpose(psk[:], Kh[:, t, :], ident16[:])
                nc.vector.tensor_copy(KT[0:64, ts_], psk[:])

            def stage_q(t):
                ts_ = slice(t * 128, (t + 1) * 128)
                psq = psSp.tile([128, 128], F16, name="psq", tag="ps")
                for h in range(HPC):
                    nc.tensor.transpose(
                        psq[64 * h : 64 * (h + 1), :],
                        Qh[:, h, t, :],
                        ident16[:],
                        tile_position=(0, 64 * h),
                    )
                nc.vector.tensor_copy(QT[:, ts_], psq[:])

            for t in range(NKT):
                stage_k(t)
            nc.sync.dma_start(out=KT[64:128, :], in_=KT[0:64, :])
            for t in range(4):
                stage_q(t)

            # ---- flattened main loop over (pass, kt) ----
            sc = {}    # i -> score psum tile
            pt = {}    # i -> prob sbuf tile
            ps_o = {}  # pass -> psO tile [65, 2, 512]

            def emit_mm1(i):
                if i >= NIT:
                    return
                p, kt = divmod(i, NKT)
                qs = slice(p * QB, (p + 1) * QB)
                ks = slice(kt * KT_TILE, (kt + 1) * KT_TILE)
                ps_s = psSp.tile([128, HPC, QB], F32, name="ps_s", tag="ps")
                for h in range(HPC):
                    nc.tensor.matmul(
                        ps_s[:, h, :],
                        lhsT=KT[64 * h : 64 * (h + 1), ks],
                        rhs=QT[64 * h : 64 * (h + 1), qs],
                        start=True,
                        stop=True,
                    )
                sc[i] = ps_s

            def emit_act(i):
                ptile = ptp.tile([128, HPC, QB], F16, name="ptile")
                nc.scalar.activation(
                    ptile[:],
                    sc.pop(i)[:],
                    mybir.ActivationFunctionType.Exp,
                    scale=float(SCALE),
                )
                pt[i] = ptile

            def emit_mm2(i):
                p, kt = divmod(i, NKT)
                if kt == 0:
                    ps_o[p] = psOp.tile([D + 1, HPC, QB], F32, name="psO", tag="psO")
                ptile = pt.pop(i)
                if MERGED_MM2:
                    nc.tensor.matmul(
                        ps_o[p][:],
                        lhsT=Vt[:, kt, :],
                        rhs=ptile[:],
                        start=(kt == 0),
                        stop=(kt == NKT - 1),
                    )
                else:
                    for h in range(HPC):
                        nc.tensor.matmul(
                            ps_o[p][:, h, :],
                            lhsT=Vt[:, kt, :],
                            rhs=ptile[:, h, :],
                            start=(kt == 0),
                            stop=(kt == NKT - 1),
                        )

            def emit_drain(p):
                qs = slice(p * QB, (p + 1) * QB)
                ots = []
                # h0's copy on DVE, h1's on the scalar engine (which idles at
                # the pass boundary anyway) so the psO WAR clears ~700ns
                # earlier for the next pass's first MM2.
                for h in range(HPC):
                    ot = otp.tile([D + 1, QB], F16, name="ot")
                    if h == 0:
                        nc.vector.tensor_copy(ot[:], ps_o[p][:, h, :])
                    else:
                        nc.scalar.copy(ot[:], ps_o[p][:, h, :])
                    ots.append(ot)
                for h in range(HPC):
                    ps_t = psSp.tile([128, QB // 128, D + 2], F16, name="ps_t", tag="ps")
                    for j in range(QB // 128):
                        nc.tensor.transpose(
                            ps_t[:, j, 0 : D + 1],
                            ots[h][:, j * 128 : (j + 1) * 128],
                            ident16[0 : D + 1, 0 : D + 1],
                        )
                    rec = recp.tile([128, QB // 128, 1], F32, name="rec")
                    nc.vector.reciprocal(rec[:], ps_t[:, :, D : D + 1])
                    outsb = outp.tile([128, QB // 128, D], F32, name="outsb")
                    for j in range(QB // 128):
                        nc.vector.tensor_scalar_mul(
                            outsb[:, j, :], ps_t[:, j, 0:D], rec[:, j, :]
                        )
                    nc.sync.dma_start(
                        out=Od.ap()[h, qs, :].rearrange("(j p) d -> p j d", p=128),
                        in_=outsb[:],
                    )
                del ps_o[p]

            emit_mm1(0)
            emit_mm1(1)
            for i in range(NIT):
                p, kt = divmod(i, NKT)
                emit_mm1(i + 2)
                # Stage the remaining Q tiles during pass 0 (PE work that
                # fills dependence stalls while the pipeline warms up).
                if p == 0 and 2 <= kt <= 13 and NQB > 1:
                    stage_q(2 + kt)
                # Dependency-free PE filler: a junk weight load sized so the
                # PE stream stays gapless at the ACT-bound cadence.  Without
                # it the tensor engine's DVFS ramp (2.4GHz needs ~3us of
                # continuous busy) keeps resetting on the ACT->MM2 waits.
                for _ in range(FILLER_N):
                    nc.tensor.ldweights(KT[0:64, 0:128])
                emit_act(i)
                emit_mm2(i)
                if kt == NKT - 1:
                    emit_drain(p)
    nc.compile()
    return nc


_CACHED = {}


def _get_nc():
    if "nc" not in _CACHED:
        _CACHED["nc"] = build_nc()
    return _CACHED["nc"]


def _shard(Q, K, V):
    in_maps = []
    for c in range(NCORES):
        b = c // 4
        h0 = (c % 4) * HPC
        in_maps.append(
            {
                "q": np.ascontiguousarray(np.asarray(Q, np.float32)[b, h0 : h0 + HPC]),
                "k": np.ascontiguousarray(np.asarray(K, np.float32)[b, 0]),
                "v": np.ascontiguousarray(np.asarray(V, np.float32)[b, 0]),
            }
        )
    return in_maps


def kernel(Q, K, V, trace=False):
    nc = _get_nc()
    res = run_bass_kernel_spmd(nc, _shard(Q, K, V), list(range(NCORES)), trace=trace)
    _CACHED["last_result"] = res
    O = np.empty((B, H, S, D), np.float32)
    for c, r in enumerate(res.results):
        b = c // 4
        h0 = (c % 4) * HPC
        O[b, h0 : h0 + HPC] = r["o"]
    return O
